# revision 1
# baseline (speedup 1.0000x reference)
"""Trainium2 Bass kernel for windowed sparse attention (nn_Attention_74938589380827).

Math (per reference):
  q = seq @ Wq.T + bq ; k,v = split(seq @ Wkv.T) ; heads h=8, dh=64
  windows of w=128 tokens; context per window = 4 memory slots + prev window + cur window
  sim = softclamp_50(q*dh^-0.5 @ k.T + bias) ; masked -> -inf ; softmax ; @ v
  out gated by sigmoid(seq @ Wg.T + bg), then @ Wo.T

Sharding: sequence-parallel over 8 cores: core c -> batch c//4, token range
[1024*(c%4), 1024*(c%4+1)) = 8 windows. Each core needs one extra window of
k/v lookback (host ships a 1152-token transposed seq slice; zeros for core 0
of each batch, whose first window has no predecessor -- its mask bits kill it).

Host-side prep (sharding/layout only, no math): slices, transposes,
bias+mask fold into an additive -1e30 tensor (select, pre-softclamp order
preserved because masked lanes saturate tanh to -50 then underflow exp).

Device per core:
  phase B: projections q,k (transposed layout, fp16), v (natural, fp16),
           gate logits tanh(g/2) (fp32) -- all via fp32r matmuls from DMA'd
           fp32r weights/activations; bg folded in via K=1 matmul.
  phase C: per window i, per head h: sim (fp16 matmul, fp32 psum) -> +beff
           (DVE) -> tanh (ACT) -> exp*50 w/ row-sum accum (ACT, fp16 out) ->
           DMA-transpose attn -> attn@v (fp16) into per-window psum ->
           gate*recip fold (DVE) -> DMA-transpose -> @WoT (fp16) -> out.
"""
import numpy as np
import concourse.bass as bass
import concourse.tile as tile
from concourse import mybir
from concourse.bass_utils import run_bass_kernel_spmd

F32 = mybir.dt.float32
F32R = mybir.dt.float32r
F16 = mybir.dt.float16
A = mybir.ActivationFunctionType

HEADS, DH, W, M = 8, 64, 128, 4
B, N, DIM = 2, 4096, 512
NW_CORE = 8                      # windows per core
TLOC = NW_CORE * W + W           # 1152 tokens incl. lookback window
NEG = -1.0e30
SCALE = DH ** -0.5


def _split_sync_waits(nc):
    """This container's walrus accepts only one sync-wait per instruction;
    hoist extra waits onto same-engine NoOps placed just before."""
    k = 0
    for f in nc.m.functions:
        for b in f.blocks:
            out = []
            for inst in b.instructions:
                si = inst.sync_info
                if si is not None and len(si.on_wait) > 1:
                    waits = list(si.on_wait)
                    for w in waits[:-1]:
                        k += 1
                        out.append(mybir.InstNoOp(
                            name=f"I-wsplit-{k}",
                            sync_info=mybir.SyncInfo(on_wait=[w], on_update=[]),
                            bass_nofuse=True,
                            engine=inst.engine,
                        ))
                    inst.sync_info = mybir.SyncInfo(
                        on_wait=[waits[-1]], on_update=list(si.on_update))
                out.append(inst)
            b.instructions = out


def _build_program():
    nc = bass.Bass()
    seqT = nc.declare_dram_parameter("seqT", [DIM, TLOC], F32R, isOutput=False)
    beff = nc.declare_dram_parameter("beff", [NW_CORE, W, M + 2 * W], F32, isOutput=False)
    WqT = nc.declare_dram_parameter("WqT", [DIM, DIM], F32R, isOutput=False)
    WkvT = nc.declare_dram_parameter("WkvT", [DIM, 2 * DIM], F32R, isOutput=False)
    WgT = nc.declare_dram_parameter("WgT", [DIM, DIM], F32R, isOutput=False)
    WoT = nc.declare_dram_parameter("WoT", [DIM, DIM], F16, isOutput=False)
    bqs = nc.declare_dram_parameter("bqs", [4, 128], F32, isOutput=False)
    bgT = nc.declare_dram_parameter("bgT", [1, DIM], F32R, isOutput=False)
    ones = nc.declare_dram_parameter("ones", [1, 128], F32R, isOutput=False)
    mkT = nc.declare_dram_parameter("mkT", [128, 4, M], F16, isOutput=False)
    memv = nc.declare_dram_parameter("memv", [M, DIM], F16, isOutput=False)
    y = nc.declare_dram_parameter("y", [NW_CORE * W, DIM], F32, isOutput=True)

    CTX = M + 2 * W  # 260

    with tile.TileContext(nc) as tc:
        from contextlib import ExitStack
        with ExitStack() as ctx:
            cst = ctx.enter_context(tc.tile_pool(name="cst", bufs=1))
            acts = ctx.enter_context(tc.tile_pool(name="acts", bufs=1))
            win = ctx.enter_context(tc.tile_pool(name="win", bufs=3))
            wk = ctx.enter_context(tc.tile_pool(name="wk", bufs=3))
            att = ctx.enter_context(tc.tile_pool(name="att", bufs=4))

            seqT_sb = cst.tile([128, 4, TLOC], F32R)
            WqT_sb = cst.tile([128, 4, DIM], F32R)
            WkvT_sb = cst.tile([128, 4, 2 * DIM], F32R)
            WgT_sb = cst.tile([128, 4, DIM], F32R)
            WoT_sb = cst.tile([128, 4, DIM], F16)
            bqs_sb = cst.tile([128, 4], F32)
            bgT_sb = cst.tile([1, DIM], F32R)
            ones_sb = cst.tile([1, 128], F32R)
            mkT_sb = cst.tile([128, 4, M], F16)
            memv_sb = cst.tile([M, DIM], F16)

            nc.sync.dma_start(out=WqT_sb[:], in_=WqT.ap().rearrange("(c p) n -> p c n", p=128))
            nc.sync.dma_start(out=WkvT_sb[:], in_=WkvT.ap().rearrange("(c p) n -> p c n", p=128))
            nc.sync.dma_start(out=WgT_sb[:], in_=WgT.ap().rearrange("(c p) n -> p c n", p=128))
            nc.sync.dma_start(out=WoT_sb[:], in_=WoT.ap().rearrange("(c p) n -> p c n", p=128))
            nc.sync.dma_start(out=seqT_sb[:], in_=seqT.ap().rearrange("(c p) t -> p c t", p=128))
            nc.sync.dma_start(out=bqs_sb[:], in_=bqs.ap().rearrange("c p -> p c"))
            nc.sync.dma_start(out=bgT_sb[:], in_=bgT[:])
            nc.sync.dma_start(out=ones_sb[:], in_=ones[:])
            nc.sync.dma_start(out=mkT_sb[:], in_=mkT[:])
            nc.sync.dma_start(out=memv_sb[:], in_=memv[:])

            qT_sb = acts.tile([128, 4, NW_CORE * W], F16)     # [di, t] t=local-128
            kT_sb = acts.tile([128, 4, TLOC], F16)            # [di, t]
            v_sb = acts.tile([128, 9, DIM], F16)              # [t-tile, di]
            th_sb = acts.tile([128, NW_CORE, DIM], F32)       # tanh(g/2), [t-tile, di]

            with tc.tile_pool(name="psB", bufs=4, space="PSUM") as psB:
                # q: [di, t] layout, scaled by dh^-0.5, bias folded
                for m in range(4):
                    for th in range(2):
                        ps = psB.tile([128, 512], F32, tag="ps")
                        for c in range(4):
                            nc.tensor.matmul(
                                ps[:],
                                WqT_sb[:, c, m * 128:(m + 1) * 128],
                                seqT_sb[:, c, W + th * 512: W + (th + 1) * 512],
                                start=(c == 0), stop=(c == 3))
                        nc.scalar.activation(
                            qT_sb[:, m, th * 512:(th + 1) * 512], ps[:],
                            A.Identity, scale=SCALE, bias=bqs_sb[:, m:m + 1])
                # k: [di, t] layout
                for m in range(4):
                    for t0, t1 in ((0, 512), (512, 1024), (1024, TLOC)):
                        ps = psB.tile([128, 512], F32, tag="ps")
                        for c in range(4):
                            nc.tensor.matmul(
                                ps[:, :t1 - t0],
                                WkvT_sb[:, c, m * 128:(m + 1) * 128],
                                seqT_sb[:, c, t0:t1],
                                start=(c == 0), stop=(c == 3))
                        nc.vector.tensor_copy(kT_sb[:, m, t0:t1], ps[:, :t1 - t0])
                # v: natural [t, di] layout
                for tt in range(9):
                    ps = psB.tile([128, 512], F32, tag="ps")
                    for c in range(4):
                        nc.tensor.matmul(
                            ps[:],
                            seqT_sb[:, c, tt * 128:(tt + 1) * 128],
                            WkvT_sb[:, c, DIM:2 * DIM],
                            start=(c == 0), stop=(c == 3))
                    nc.vector.tensor_copy(v_sb[:, tt, :], ps[:])
                # gate logits -> tanh(x/2); bg via K=1 matmul
                for tt in range(NW_CORE):
                    ps = psB.tile([128, 512], F32, tag="ps")
                    for c in range(4):
                        nc.tensor.matmul(
                            ps[:],
                            seqT_sb[:, c, W + tt * 128: W + (tt + 1) * 128],
                            WgT_sb[:, c, :],
                            start=(c == 0), stop=False)
                    nc.tensor.matmul(ps[:], ones_sb[0:1, :], bgT_sb[0:1, :],
                                     start=False, stop=True)
                    nc.scalar.activation(th_sb[:, tt, :], ps[:], A.Tanh, scale=0.5)

            with tc.tile_pool(name="psS", bufs=3, space="PSUM") as psS, \
                 tc.tile_pool(name="psO", bufs=2, space="PSUM") as psO, \
                 tc.tile_pool(name="psY", bufs=2, space="PSUM") as psY:
                for i in range(NW_CORE):
                    beff_sb = win.tile([128, CTX], F32, tag="beff")
                    nc.sync.dma_start(out=beff_sb[:], in_=beff[i])
                    rs_sb = win.tile([128, 8], F32, tag="rs")
                    hrec_sb = win.tile([128, 8], F32, tag="hrec")
                    gr_sb = win.tile([128, DIM], F32, tag="gr")
                    out_ps = psO.tile([128, DIM], F32, tag="out")

                    attnTs = []
                    for h in range(HEADS):
                        hp, off = h // 2, 64 * (h % 2)
                        qsl = qT_sb[off:off + 64, hp, i * 128:(i + 1) * 128]
                        sim = psS.tile([128, CTX], F32, tag="sim")
                        nc.tensor.matmul(sim[:, 0:M], qsl,
                                         mkT_sb[off:off + 64, hp, :],
                                         start=True, stop=True)
                        nc.tensor.matmul(sim[:, M:CTX], qsl,
                                         kT_sb[off:off + 64, hp, i * 128:i * 128 + 256],
                                         start=True, stop=True)
                        s1 = wk.tile([128, CTX], F32, tag="s1")
                        nc.vector.tensor_add(s1[:], sim[:], beff_sb[:])
                        s2 = wk.tile([128, CTX], F32, tag="s2")
                        nc.scalar.activation(s2[:], s1[:], A.Tanh, scale=1.0 / 50.0)
                        p16 = wk.tile([128, CTX], F16, tag="p16")
                        nc.scalar.activation(p16[:], s2[:], A.Exp, scale=50.0,
                                             accum_out=rs_sb[:, h:h + 1])
                        attnT = att.tile([128, 3, 128], F16, tag="attnT")
                        nc.sync.dma_start_transpose(attnT[:, 0, :], p16[:, M:M + 128])
                        nc.sync.dma_start_transpose(attnT[:, 1, :], p16[:, M + 128:CTX])
                        nc.sync.dma_start_transpose(attnT[:, 2, :], p16[:, 0:128])
                        attnTs.append(attnT)

                    for h in range(HEADS):
                        attnT = attnTs[h]
                        o = h * 64
                        nc.tensor.matmul(out_ps[:, o:o + 64], attnT[0:M, 2, :],
                                         memv_sb[:, o:o + 64], start=True, stop=False)
                        nc.tensor.matmul(out_ps[:, o:o + 64], attnT[:, 0, :],
                                         v_sb[:, i, o:o + 64], start=False, stop=False)
                        nc.tensor.matmul(out_ps[:, o:o + 64], attnT[:, 1, :],
                                         v_sb[:, i + 1, o:o + 64], start=False, stop=True)

                    # gr = sigmoid(g)*recip = 0.5*tanh(g/2)*2/(2*rowsum) + 0.5/rowsum
                    nc.vector.tensor_scalar(hrec_sb[:], rs_sb[:], 2.0, None,
                                            op0=mybir.AluOpType.mult)
                    nc.vector.reciprocal(hrec_sb[:], hrec_sb[:])
                    for h in range(HEADS):
                        o = h * 64
                        nc.vector.tensor_scalar(
                            gr_sb[:, o:o + 64], th_sb[:, i, o:o + 64],
                            hrec_sb[:, h:h + 1], hrec_sb[:, h:h + 1],
                            op0=mybir.AluOpType.mult, op1=mybir.AluOpType.add)
                    og16 = win.tile([128, DIM], F16, tag="og16")
                    nc.vector.tensor_mul(og16[:], out_ps[:], gr_sb[:])
                    ogT = win.tile([128, 4, 128], F16, tag="ogT")
                    for c in range(4):
                        nc.sync.dma_start_transpose(ogT[:, c, :],
                                                    og16[:, c * 128:(c + 1) * 128])
                    y_ps = psY.tile([128, DIM], F32, tag="y")
                    for c in range(4):
                        nc.tensor.matmul(y_ps[:], ogT[:, c, :], WoT_sb[:, c, :],
                                         start=(c == 0), stop=(c == 3))
                    y_sb = win.tile([128, DIM], F32, tag="ysb")
                    nc.scalar.copy(y_sb[:], y_ps[:])
                    nc.sync.dma_start(out=y[i * 128:(i + 1) * 128, :], in_=y_sb[:])

    _split_sync_waits(nc)
    return nc


_PROGRAM = None


def _get_program():
    global _PROGRAM
    if _PROGRAM is None:
        _PROGRAM = _build_program()
    return _PROGRAM


def _host_prep(seq, mask, windowed_mask, attn_bias, Wq, bq, Wkv, Wo, Wg, bg, memory_kv):
    """Shard + lay out inputs for the 8 cores. Layout/slicing only."""
    seq = np.asarray(seq, np.float32)
    mask = np.asarray(mask, bool)
    windowed_mask = np.asarray(windowed_mask, bool)
    attn_bias = np.asarray(attn_bias, np.float32)
    Wq = np.asarray(Wq, np.float32)
    bq = np.asarray(bq, np.float32)
    Wkv = np.asarray(Wkv, np.float32)
    Wo = np.asarray(Wo, np.float32)
    Wg = np.asarray(Wg, np.float32)
    bg = np.asarray(bg, np.float32)
    memory_kv = np.asarray(memory_kv, np.float32)

    WqT = np.ascontiguousarray(Wq.T)
    WkvT = np.ascontiguousarray(Wkv.T)
    WgT = np.ascontiguousarray(Wg.T)
    WoT = np.ascontiguousarray(Wo.T).astype(np.float16)
    bqs = (bq * SCALE).reshape(4, 128)
    bgT = bg.reshape(1, DIM)
    ones = np.ones((1, 128), np.float32)
    mkT = np.zeros((128, 4, M), np.float16)
    for hp in range(4):
        for j in range(2):
            mkT[j * 64:(j + 1) * 64, hp, :] = memory_kv[0][2 * hp + j].T
    memv = memory_kv[1].transpose(1, 0, 2).reshape(M, DIM).astype(np.float16)

    nw = N // W  # 32
    in_maps = []
    for bi in range(B):
        seqTb = np.ascontiguousarray(seq[bi].T)          # [512, 4096]
        # beff for all 32 windows of this batch
        abr = attn_bias[bi].reshape(nw, W, nw, W)
        ar = np.arange(nw)
        cur = abr[ar, :, ar, :]                          # [32, W, W]
        prev = np.zeros_like(cur)
        prev[1:] = abr[ar[1:], :, ar[:-1], :]
        bias_tok = np.concatenate([prev, cur], axis=-1)  # [32, W, 2W]
        mw = mask[bi].reshape(nw, W)
        mprev = np.zeros_like(mw)
        mprev[1:] = mw[:-1]
        mcat = np.concatenate([mprev, mw], axis=-1)      # [32, 2W]
        allowed = windowed_mask[bi] & mcat[:, None, :]   # [32, W, 2W]
        beff_tok = np.where(allowed, bias_tok, NEG).astype(np.float32)
        beff_b = np.zeros((nw, W, M + 2 * W), np.float32)
        beff_b[:, :, M:] = beff_tok

        for wg in range(4):
            t0 = wg * 1024
            seqT_c = np.zeros((DIM, TLOC), np.float32)
            lo = t0 - W
            if lo < 0:
                seqT_c[:, W:] = seqTb[:, t0:t0 + 1024]
            else:
                seqT_c[:] = seqTb[:, lo:t0 + 1024]
            in_maps.append(dict(
                seqT=seqT_c,
                beff=np.ascontiguousarray(beff_b[wg * 8:(wg + 1) * 8]),
                WqT=WqT, WkvT=WkvT, WgT=WgT, WoT=WoT,
                bqs=bqs, bgT=bgT, ones=ones, mkT=mkT, memv=memv,
            ))
    return in_maps


def kernel(**inputs):
    nc = _get_program()
    in_maps = _host_prep(**inputs)
    res = run_bass_kernel_spmd(nc, in_maps, list(range(8)))
    out = np.empty((B, N, DIM), np.float32)
    for c in range(8):
        bi, wg = c // 4, c % 4
        out[bi, wg * 1024:(wg + 1) * 1024, :] = res.results[c]["y"]
    return out


# revision 3
# speedup vs baseline: 2.0067x; 2.0067x over previous
"""Trainium2 Bass kernel for windowed sparse attention (nn_Attention_74938589380827).

Math (per reference):
  q = seq @ Wq.T + bq ; k,v = split(seq @ Wkv.T) ; heads h=8, dh=64
  windows of w=128 tokens; context per window = 4 memory slots + prev window + cur window
  sim = softclamp_50(q*dh^-0.5 @ k.T + bias) ; masked -> -1e30 ; softmax ; @ v
  out gated by sigmoid(seq @ Wg.T + bg), then @ Wo.T

Sharding: sequence-parallel over 8 cores: core c -> batch c//4, token range
[1024*(c%4), 1024*(c%4+1)) = 8 windows. Each core gets one extra window of
k/v lookback (host ships a 1152-token transposed seq slice; zeros for the
first core of each batch, whose window 0 lookback is fully masked anyway).

Key layout trick: sim is computed TRANSPOSED (simT[j, t] = k_j . q_t) so the
softmax numerator exp(softclamp(simT+beffT)) lands in SBUF already in the
[contraction, out] layout that the attn@v matmul needs as lhsT -- no
per-head transposes anywhere. Row sums are N=1 matmuls (expT.T @ ones) that
land as [t, 1] per-partition scalars; the reciprocal is folded into the
gate multiplier, and sigmoid(x) = 0.5*tanh(x/2) + 0.5 is folded there too
(keeps the whole kernel on one ACT table: exp/tanh/identity/copy).

Host-side prep (sharding/layout only): slices, transposes, bias+mask fold
into an additive -1e30 tensor (select preserves pre-softclamp order because
masked lanes saturate tanh to -50 and underflow exp).
"""
import numpy as np
import concourse.bass as bass
import concourse.tile as tile
from concourse import mybir
from concourse.bass_utils import run_bass_kernel_spmd

F32 = mybir.dt.float32
F32R = mybir.dt.float32r
F16 = mybir.dt.float16
A = mybir.ActivationFunctionType
OP = mybir.AluOpType

HEADS, DH, W, M = 8, 64, 128, 4
B, N, DIM = 2, 4096, 512
NW_CORE = 8                      # windows per core
TLOC = NW_CORE * W + W           # 1152 tokens incl. lookback window
NEG = -1.0e30
SCALE = DH ** -0.5
SIMW = 3 * W                     # simT tile free size: [prev t | cur t | mem t]


def _split_sync_waits(nc):
    """This container's walrus accepts only one sync-wait per instruction;
    hoist extra waits onto same-engine NoOps placed just before."""
    k = 0
    for f in nc.m.functions:
        for b in f.blocks:
            out = []
            for inst in b.instructions:
                si = inst.sync_info
                if si is not None and len(si.on_wait) > 1:
                    waits = list(si.on_wait)
                    for w in waits[:-1]:
                        k += 1
                        out.append(mybir.InstNoOp(
                            name=f"I-wsplit-{k}",
                            sync_info=mybir.SyncInfo(on_wait=[w], on_update=[]),
                            bass_nofuse=True,
                            engine=inst.engine,
                        ))
                    inst.sync_info = mybir.SyncInfo(
                        on_wait=[waits[-1]], on_update=list(si.on_update))
                out.append(inst)
            b.instructions = out


def _build_program():
    nc = bass.Bass()
    seqT = nc.declare_dram_parameter("seqT", [DIM, TLOC], F32R, isOutput=False)
    beffT = nc.declare_dram_parameter("beffT", [NW_CORE, W, SIMW], F32, isOutput=False)
    WqT = nc.declare_dram_parameter("WqT", [DIM, DIM], F32R, isOutput=False)
    WkvT = nc.declare_dram_parameter("WkvT", [DIM, 2 * DIM], F32R, isOutput=False)
    WgT = nc.declare_dram_parameter("WgT", [DIM, DIM], F32R, isOutput=False)
    WoT = nc.declare_dram_parameter("WoT", [DIM, DIM], F16, isOutput=False)
    bqs = nc.declare_dram_parameter("bqs", [4, 128], F32, isOutput=False)
    bgT = nc.declare_dram_parameter("bgT", [1, DIM], F32R, isOutput=False)
    ones = nc.declare_dram_parameter("ones", [1, 128], F32R, isOutput=False)
    mkT = nc.declare_dram_parameter("mkT", [128, 4, M], F16, isOutput=False)
    memv = nc.declare_dram_parameter("memv", [M, DIM], F16, isOutput=False)
    y = nc.declare_dram_parameter("y", [NW_CORE * W, DIM], F32, isOutput=True)

    with tile.TileContext(nc) as tc:
        from contextlib import ExitStack
        with ExitStack() as ctx:
            cst = ctx.enter_context(tc.tile_pool(name="cst", bufs=1))
            acts = ctx.enter_context(tc.tile_pool(name="acts", bufs=1))
            win = ctx.enter_context(tc.tile_pool(name="win", bufs=3))
            wk = ctx.enter_context(tc.tile_pool(name="wk", bufs=4))

            seqT_sb = cst.tile([128, 4, TLOC], F32R)
            WqT_sb = cst.tile([128, 4, DIM], F32R)
            WkvT_sb = cst.tile([128, 4, 2 * DIM], F32R)
            WgT_sb = cst.tile([128, 4, DIM], F32R)
            WoT_sb = cst.tile([128, 4, DIM], F16)
            bqs_sb = cst.tile([128, 4], F32)
            bgT_sb = cst.tile([1, DIM], F32R)
            ones_sb = cst.tile([1, 128], F32R)
            mkT_sb = cst.tile([128, 4, M], F16)
            memv_sb = cst.tile([M, DIM], F16)
            ones16_sb = cst.tile([128, 1], F16)
            nc.vector.memset(ones16_sb[:], 1.0)

            # bulk loads on the SWDGE queue (gpsimd dispatch is cheap);
            # the sync sequencer is reserved for the few DMA transposes.
            nc.gpsimd.dma_start(out=WqT_sb[:], in_=WqT.ap().rearrange("(c p) n -> p c n", p=128))
            nc.gpsimd.dma_start(out=WkvT_sb[:], in_=WkvT.ap().rearrange("(c p) n -> p c n", p=128))
            nc.gpsimd.dma_start(out=WgT_sb[:], in_=WgT.ap().rearrange("(c p) n -> p c n", p=128))
            nc.gpsimd.dma_start(out=WoT_sb[:], in_=WoT.ap().rearrange("(c p) n -> p c n", p=128))
            nc.gpsimd.dma_start(out=seqT_sb[:], in_=seqT.ap().rearrange("(c p) t -> p c t", p=128))
            nc.gpsimd.dma_start(out=bqs_sb[:], in_=bqs.ap().rearrange("c p -> p c"))
            nc.gpsimd.dma_start(out=bgT_sb[:], in_=bgT[:])
            nc.gpsimd.dma_start(out=ones_sb[:], in_=ones[:])
            nc.gpsimd.dma_start(out=mkT_sb[:], in_=mkT[:])
            nc.gpsimd.dma_start(out=memv_sb[:], in_=memv[:])

            qT_sb = acts.tile([128, 4, NW_CORE * W], F16)     # [di, t]
            kT_sb = acts.tile([128, 4, TLOC], F16)            # [di, t]
            v_sb = acts.tile([128, 9, DIM], F16)              # [t-tile, di]
            th_sb = acts.tile([128, NW_CORE, DIM], F32)       # tanh(g/2)

            with tc.tile_pool(name="psB", bufs=4, space="PSUM") as psB:
                # q: [di, t] layout, scaled by dh^-0.5, bias folded (DVE)
                for m in range(4):
                    for th in range(2):
                        ps = psB.tile([128, 512], F32, tag="ps")
                        for c in range(4):
                            nc.tensor.matmul(
                                ps[:],
                                WqT_sb[:, c, m * 128:(m + 1) * 128],
                                seqT_sb[:, c, W + th * 512: W + (th + 1) * 512],
                                start=(c == 0), stop=(c == 3))
                        nc.vector.tensor_scalar(
                            qT_sb[:, m, th * 512:(th + 1) * 512], ps[:],
                            SCALE, bqs_sb[:, m:m + 1], op0=OP.mult, op1=OP.add)
                # k: [di, t] layout
                for m in range(4):
                    for t0, t1 in ((0, 512), (512, 1024), (1024, TLOC)):
                        ps = psB.tile([128, 512], F32, tag="ps")
                        for c in range(4):
                            nc.tensor.matmul(
                                ps[:, :t1 - t0],
                                WkvT_sb[:, c, m * 128:(m + 1) * 128],
                                seqT_sb[:, c, t0:t1],
                                start=(c == 0), stop=(c == 3))
                        nc.vector.tensor_copy(kT_sb[:, m, t0:t1], ps[:, :t1 - t0])
                # v: natural [t, di] layout
                for tt in range(9):
                    ps = psB.tile([128, 512], F32, tag="ps")
                    for c in range(4):
                        nc.tensor.matmul(
                            ps[:],
                            seqT_sb[:, c, tt * 128:(tt + 1) * 128],
                            WkvT_sb[:, c, DIM:2 * DIM],
                            start=(c == 0), stop=(c == 3))
                    nc.vector.tensor_copy(v_sb[:, tt, :], ps[:])
                # gate logits -> tanh(x/2); bg via K=1 matmul
                for tt in range(NW_CORE):
                    ps = psB.tile([128, 512], F32, tag="ps")
                    for c in range(4):
                        nc.tensor.matmul(
                            ps[:],
                            seqT_sb[:, c, W + tt * 128: W + (tt + 1) * 128],
                            WgT_sb[:, c, :],
                            start=(c == 0), stop=False)
                    nc.tensor.matmul(ps[:], ones_sb[0:1, :], bgT_sb[0:1, :],
                                     start=False, stop=True)
                    nc.scalar.activation(th_sb[:, tt, :], ps[:], A.Tanh, scale=0.5)

            with tc.tile_pool(name="psS", bufs=3, space="PSUM") as psS, \
                 tc.tile_pool(name="psO", bufs=2, space="PSUM") as psO, \
                 tc.tile_pool(name="psR", bufs=1, space="PSUM") as psR, \
                 tc.tile_pool(name="psY", bufs=2, space="PSUM") as psY:
                for i in range(NW_CORE):
                    beffT_sb = win.tile([128, SIMW], F32, tag="beff")
                    nc.gpsimd.dma_start(out=beffT_sb[:], in_=beffT[i])
                    hrec_sb = win.tile([128, 8], F32, tag="hrec")
                    gr_sb = win.tile([128, DIM], F32, tag="gr")
                    out_ps = psO.tile([128, DIM], F32, tag="out")
                    rsT_ps = psR.tile([128, 8], F32, tag="rs")

                    for h in range(HEADS):
                        hp, off = h // 2, 64 * (h % 2)
                        qsl = qT_sb[off:off + 64, hp, i * 128:(i + 1) * 128]
                        # simT: [j, t] -- prev | cur | mem(rows 0:4)
                        simT = psS.tile([128, SIMW], F32, tag="sim")
                        nc.tensor.matmul(
                            simT[:, 0:128],
                            kT_sb[off:off + 64, hp, i * 128:(i + 1) * 128],
                            qsl, start=True, stop=True)
                        nc.tensor.matmul(
                            simT[:, 128:256],
                            kT_sb[off:off + 64, hp, (i + 1) * 128:(i + 2) * 128],
                            qsl, start=True, stop=True)
                        nc.tensor.matmul(
                            simT[0:M, 256:384],
                            mkT_sb[off:off + 64, hp, :],
                            qsl, start=True, stop=True)
                        s1 = wk.tile([128, SIMW], F32, tag="s1")
                        nc.vector.tensor_add(s1[:], simT[:], beffT_sb[:])
                        s2 = wk.tile([128, SIMW], F32, tag="s2")
                        nc.scalar.activation(s2[:], s1[:], A.Tanh, scale=1.0 / 50.0)
                        et = wk.tile([128, SIMW], F16, tag="et")
                        nc.scalar.activation(et[:], s2[:], A.Exp, scale=50.0)
                        o = h * 64
                        # attn @ v, unnormalized (lhsT = expT directly)
                        nc.tensor.matmul(out_ps[:, o:o + 64], et[:, 0:128],
                                         v_sb[:, i, o:o + 64], start=True, stop=False)
                        nc.tensor.matmul(out_ps[:, o:o + 64], et[:, 128:256],
                                         v_sb[:, i + 1, o:o + 64], start=False, stop=False)
                        nc.tensor.matmul(out_ps[:, o:o + 64], et[0:M, 256:384],
                                         memv_sb[:, o:o + 64], start=False, stop=True)
                        # row sums as [t, 1] via N=1 matmuls
                        nc.tensor.matmul(rsT_ps[:, h:h + 1], et[:, 0:128],
                                         ones16_sb[:], start=True, stop=False)
                        nc.tensor.matmul(rsT_ps[:, h:h + 1], et[:, 128:256],
                                         ones16_sb[:], start=False, stop=False)
                        nc.tensor.matmul(rsT_ps[:, h:h + 1], et[0:M, 256:384],
                                         ones16_sb[0:M, :], start=False, stop=True)

                    # gr = sigmoid(g)/rowsum = (0.5*tanh(g/2) + 0.5) / rowsum
                    nc.vector.tensor_scalar(hrec_sb[:], rsT_ps[:], 2.0, None, op0=OP.mult)
                    nc.vector.reciprocal(hrec_sb[:], hrec_sb[:])
                    for h in range(HEADS):
                        o = h * 64
                        nc.vector.tensor_scalar(
                            gr_sb[:, o:o + 64], th_sb[:, i, o:o + 64],
                            hrec_sb[:, h:h + 1], hrec_sb[:, h:h + 1],
                            op0=OP.mult, op1=OP.add)
                    og16 = win.tile([128, DIM], F16, tag="og16")
                    nc.vector.tensor_mul(og16[:], out_ps[:], gr_sb[:])
                    ogT = win.tile([128, 4, 128], F16, tag="ogT")
                    for c in range(4):
                        nc.sync.dma_start_transpose(ogT[:, c, :],
                                                    og16[:, c * 128:(c + 1) * 128])
                    y_ps = psY.tile([128, DIM], F32, tag="y")
                    for c in range(4):
                        nc.tensor.matmul(y_ps[:], ogT[:, c, :], WoT_sb[:, c, :],
                                         start=(c == 0), stop=(c == 3))
                    y_sb = win.tile([128, DIM], F32, tag="ysb")
                    nc.scalar.copy(y_sb[:], y_ps[:])
                    nc.gpsimd.dma_start(out=y[i * 128:(i + 1) * 128, :], in_=y_sb[:])

    _split_sync_waits(nc)
    return nc


_PROGRAM = None


def _get_program():
    global _PROGRAM
    if _PROGRAM is None:
        _PROGRAM = _build_program()
    return _PROGRAM


def _host_prep(seq, mask, windowed_mask, attn_bias, Wq, bq, Wkv, Wo, Wg, bg, memory_kv):
    """Shard + lay out inputs for the 8 cores. Layout/slicing only."""
    seq = np.asarray(seq, np.float32)
    mask = np.asarray(mask, bool)
    windowed_mask = np.asarray(windowed_mask, bool)
    attn_bias = np.asarray(attn_bias, np.float32)
    Wq = np.asarray(Wq, np.float32)
    bq = np.asarray(bq, np.float32)
    Wkv = np.asarray(Wkv, np.float32)
    Wo = np.asarray(Wo, np.float32)
    Wg = np.asarray(Wg, np.float32)
    bg = np.asarray(bg, np.float32)
    memory_kv = np.asarray(memory_kv, np.float32)

    WqT = np.ascontiguousarray(Wq.T)
    WkvT = np.ascontiguousarray(Wkv.T)
    WgT = np.ascontiguousarray(Wg.T)
    WoT = np.ascontiguousarray(Wo.T).astype(np.float16)
    bqs = (bq * SCALE).reshape(4, 128)
    bgT = bg.reshape(1, DIM)
    ones = np.ones((1, 128), np.float32)
    mkT = np.zeros((128, 4, M), np.float16)
    for hp in range(4):
        for j in range(2):
            mkT[j * 64:(j + 1) * 64, hp, :] = memory_kv[0][2 * hp + j].T
    memv = memory_kv[1].transpose(1, 0, 2).reshape(M, DIM).astype(np.float16)

    nw = N // W  # 32
    in_maps = []
    for bi in range(B):
        seqTb = np.ascontiguousarray(seq[bi].T)          # [512, 4096]
        abr = attn_bias[bi].reshape(nw, W, nw, W)
        ar = np.arange(nw)
        cur = abr[ar, :, ar, :]                          # [32, W, W]
        prev = np.zeros_like(cur)
        prev[1:] = abr[ar[1:], :, ar[:-1], :]
        bias_tok = np.concatenate([prev, cur], axis=-1)  # [32, W, 2W]
        mw = mask[bi].reshape(nw, W)
        mprev = np.zeros_like(mw)
        mprev[1:] = mw[:-1]
        mcat = np.concatenate([mprev, mw], axis=-1)      # [32, 2W]
        allowed = windowed_mask[bi] & mcat[:, None, :]   # [32, W, 2W]
        beff_tok = np.where(allowed, bias_tok, NEG).astype(np.float32)
        # transposed layout: [j, prev-t | cur-t | mem-t]
        beffT_b = np.full((nw, W, SIMW), NEG, np.float32)
        beffT_b[:, :, 0:128] = beff_tok[:, :, 0:128].transpose(0, 2, 1)
        beffT_b[:, :, 128:256] = beff_tok[:, :, 128:256].transpose(0, 2, 1)
        beffT_b[:, 0:M, 256:384] = 0.0

        for wg in range(4):
            t0 = wg * 1024
            seqT_c = np.zeros((DIM, TLOC), np.float32)
            lo = t0 - W
            if lo < 0:
                seqT_c[:, W:] = seqTb[:, t0:t0 + 1024]
            else:
                seqT_c[:] = seqTb[:, lo:t0 + 1024]
            in_maps.append(dict(
                seqT=seqT_c,
                beffT=np.ascontiguousarray(beffT_b[wg * 8:(wg + 1) * 8]),
                WqT=WqT, WkvT=WkvT, WgT=WgT, WoT=WoT,
                bqs=bqs, bgT=bgT, ones=ones, mkT=mkT, memv=memv,
            ))
    return in_maps


def kernel(**inputs):
    nc = _get_program()
    in_maps = _host_prep(**inputs)
    res = run_bass_kernel_spmd(nc, in_maps, list(range(8)))
    out = np.empty((B, N, DIM), np.float32)
    for c in range(8):
        bi, wg = c // 4, c % 4
        out[bi, wg * 1024:(wg + 1) * 1024, :] = res.results[c]["y"]
    return out
